# revision 18
# baseline (speedup 1.0000x reference)
"""Trainium2 Bass kernel for nn_LocalEnergy (protein local-energy GNN).

kernel(**inputs) takes FULL unsharded inputs (B=128), shards B across 8
NeuronCores (16 samples/core, pure data parallel), runs one Bass kernel
SPMD, gathers per-core [16] energies into the full [128] output.

Layout (measured ~178us HW vs 486us baseline):
 - Host prep (indexing/layout only): embedding gather emb[seq] replicated
   into 4 shifted row-blocks + a ones row (folds b1 into the W1 matmul)
   -> SE [16, 65, 2048] fp16 per core; R transposed to [3, 16, L]; W1/W2
   packed into one fp16 const block with feature-row remapping, torsion
   sin-row and angle cos-row sign flips, and the 3.8 length-centering
   folded into b1_fl.
 - Device phase 1 (geometry): fp16 vector math in a [48, n] (rows 16c+s)
   layout, coordinate rotations and +1 shifts via DMA split across both
   HWDGE queues (no gpsimd), fp16 selection matmuls summing the 3 coords,
   Ln/Exp on the scalar engine, torsion via y = -|b2| (b1.n2). Produces
   F [128, L] fp16 with blocks [len-3.8 | -cos_theta | sin' | cos] at
   rows 32f+s.
 - Device phase 2 (MLPs): per sample, stack [69, L] fp16 = SE rows 0-64 +
   4 feature rows (early features at 65-66 so fl/ft start during the
   torsion tail; per-MLP contraction depth K=66/67/69). W1 matmuls per
   512-chunk into [H,2,512] PSUM pairs, Relu on the vector engine per
   1024-col half, W2 matmuls into double-buffered [H,2,512] PSUM, and a
   scalar-engine Relu+bias+accum_out per (sample, MLP, half). W1(s) and
   W2(s-1) emission interleaved per-MLP to keep the PE dense; SE loads
   prefetched 2 samples ahead.
"""

import sys
import types
import numpy as np
from contextlib import ExitStack


def ensure_axon_hooks():
    """The container's antenv is a stub without axon_hooks; inject it so
    run_bass_kernel_spmd(trace=True) can NTFF-profile."""
    if "antenv.axon_hooks" in sys.modules:
        return
    import antenv

    hooks = types.ModuleType("antenv.axon_hooks")
    hooks._h = None

    def set_axon_ntff_profile_hook(h):
        hooks._h = h

    def get_axon_ntff_profile_hook():
        return hooks._h

    hooks.set_axon_ntff_profile_hook = set_axon_ntff_profile_hook
    hooks.get_axon_ntff_profile_hook = get_axon_ntff_profile_hook
    sys.modules["antenv.axon_hooks"] = hooks
    antenv.axon_hooks = hooks
    try:
        from trn_agent_boot.trn_boot import _ntff_profile_via_ctypes

        hook = _ntff_profile_via_ctypes("/opt/axon/libaxon_pjrt.so")
        if hook is not None:
            set_axon_ntff_profile_hook(hook)
    except Exception:
        pass


ensure_axon_hooks()

import concourse.bass as bass  # noqa: E402
import concourse.tile as tile  # noqa: E402
from concourse import mybir, bacc, bass_utils  # noqa: E402

dt = mybir.dt
AF = mybir.ActivationFunctionType
ALU = mybir.AluOpType
AX = mybir.AxisListType

NCORES = 8
B, L, NAA, E, H = 128, 2048, 20, 16, 128
BPC = B // NCORES
W = 512
NCH = L // W                       # 4 chunks of 512 per sample
NL, NT, NP = L - 1, L - 2, L - 3
KROWS = 69                         # stack rows: 64 emb-shift + ones + 4 features
SINV = 1.0 / 16.0                  # cross-product scaling to stay in fp16 range

MLPS = ("fl", "ft", "fp")
W1K = {"fl": 66, "ft": 67, "fp": 69}   # stack rows each MLP reads
KOFF = {"fl": 1, "ft": 2, "fp": 3}   # valid cols per sample = L - KOFF


def build_nc(bpc=BPC, ll=L):
    nc = bacc.Bacc("TRN2", target_bir_lowering=False, debug=False)

    Rt_d = nc.dram_tensor("Rt", (3, bpc, ll), dt.float32, kind="ExternalInput")
    SE_d = nc.dram_tensor("SE", (bpc, 65, ll), dt.float16, kind="ExternalInput")
    S48_d = nc.dram_tensor("S48", (48, 16), dt.float16, kind="ExternalInput")
    WW_d = nc.dram_tensor("WALL", (H, 6 * H), dt.float16, kind="ExternalInput")
    BW_d = nc.dram_tensor("BW", (H, 8), dt.float32, kind="ExternalInput")
    out_d = nc.dram_tensor("out", (1, bpc), dt.float32, kind="ExternalOutput")

    nl, nt, np_ = ll - 1, ll - 2, ll - 3

    with tile.TileContext(nc) as tc, ExitStack() as ctx:
        consts = ctx.enter_context(tc.tile_pool(name="consts", bufs=1))
        # rt first: phase 1 is gated on it, so its DMAs lead the sync queue
        rt = consts.tile([48, ll], dt.float32, name="rt")
        for c in range(3):
            nc.sync.dma_start(out=rt[16 * c: 16 * c + bpc, :], in_=Rt_d.ap()[c])
        s48 = consts.tile([48, 16], dt.float16)
        nc.sync.dma_start(out=s48, in_=S48_d.ap())
        wall = consts.tile([H, 6 * H], dt.float16, name="wall")
        nc.sync.dma_start(out=wall, in_=WW_d.ap())
        bw = consts.tile([H, 8], dt.float32, name="bw")
        nc.sync.dma_start(out=bw, in_=BW_d.ap())
        w1, w2, b2c, w3c = {}, {}, {}, {}
        for j, m in enumerate(MLPS):
            w1[m] = wall[0:W1K[m], H * j: H * (j + 1)]
            w2[m] = wall[:, H * (3 + j): H * (4 + j)]
            b2c[m] = bw[:, j: j + 1]
            w3c[m] = bw[:, 3 + j: 4 + j]
        b3s = bw[0:1, 6:7]

        # feature tile: rows 32f+s, f = 0:sin' 1:cos(phi) 2:len-3.8 3:cos(theta)
        F = consts.tile([128, ll], dt.float16, name="F")
        nc.vector.memset(F, 0.0)
        eps_t = consts.tile([16, 1], dt.float32, name="eps_t")
        nc.vector.memset(eps_t, 1e-6)
        lnsinv_t = consts.tile([16, 1], dt.float32, name="lnsinv_t")
        nc.vector.memset(lnsinv_t, float(np.log(SINV)))

        acc = {}
        for m in MLPS:
            acc[m] = consts.tile([H, 2 * bpc], dt.float32, name=f"acc_{m}")

        # stack pool opens before phase 1 so SE loads prefetch under geometry
        stk = ctx.enter_context(tc.tile_pool(name="stk", bufs=6))
        stack_tiles = {}

        def alloc_stack(s):
            t = stk.tile([KROWS, ll], dt.float16, name="stack", tag="stk")
            nc.sync.dma_start(out=t[0:65, :], in_=SE_d.ap()[s])
            stack_tiles[s] = t

        for s in range(4):
            alloc_stack(s)

        # ---- geometry tiles (full-size; ops emitted per column-half) ----
        geo = ctx.enter_context(tc.tile_pool(name="geo", bufs=1))

        def g48(name, cols):
            return geo.tile([48, cols], dt.float16, name=name)

        def g16t(name, cols):
            return geo.tile([16, cols], dt.float16, name=name)

        A1 = g48("A1", nt); A2 = g48("A2", nt)
        B1 = g48("B1", nt); B2 = g48("B2", nt)
        C1s = g48("C1s", np_)
        D = g48("D", nl); D1 = g48("D1", nt)
        Ds = g48("Ds", nl); D1s = g48("D1s", nt)
        DSQ = g48("DSQ", nl); DD = g48("DD", nt)
        t_a = g48("t_a", nt); t_b = g48("t_b", nt); Cs = g48("Cs", nt)
        XR = g48("XR", np_); YR = g48("YR", np_)
        lnl = g16t("lnl", nl); rlen = g16t("rlen", nl)
        lenf = g16t("lenf", nl); lenf16 = g16t("lenf16", nl)
        rlen1 = g16t("rlen1", nt); lenf1 = g16t("lenf1", np_)
        tt1 = g16t("tt1", nt); x_sb = g16t("x_sb", np_); y_sb = g16t("y_sb", np_)
        q1 = g16t("q1", np_); q2 = g16t("q2", np_); q = g16t("q", np_)
        lnq = g16t("lnq", np_); r2v = g16t("r2v", np_)

        SP, PP = 1023, 1024      # shifted / plain column split points

        def rp(h, n):
            return (0, PP) if h == 0 else (PP, n)

        def rs(h, n):
            return (0, SP) if h == 0 else (SP, n)

        with tc.tile_pool(name="h1_ps", bufs=2, space="PSUM") as h1_ps, \
             tc.tile_pool(name="h1r_p", bufs=13) as h1r_p, \
             tc.tile_pool(name="h2_ps", bufs=2, space="PSUM") as h2_ps, \
             tc.tile_pool(name="scr_p", bufs=2) as scr_p:

            def selmm2(src, lo, hi, name):
                dst = h2_ps.tile([16, 2, W], dt.float32, name=name, tag="h2ps")
                o, c0 = 0, lo
                while c0 < hi:
                    n = min(W, hi - c0)
                    nc.tensor.matmul(dst[:, o, :n], s48, src[:, c0: c0 + n], start=True, stop=True)
                    o, c0 = o + 1, c0 + n
                return dst.rearrange("p a b -> p (a b)")

            def emit_geo_a(h):
                a, b = rp(h, nl)
                nc.vector.tensor_tensor(out=D[:, a:b], in0=rt[:, a + 1: b + 1], in1=rt[:, a:b], op=ALU.subtract)
                nc.vector.tensor_scalar(out=Ds[:, a:b], in0=D[:, a:b], scalar1=SINV, scalar2=None, op0=ALU.mult)
                nc.vector.tensor_tensor(out=DSQ[:, a:b], in0=D[:, a:b], in1=D[:, a:b], op=ALU.mult)
                a, b = rp(h, nt)
                nc.vector.tensor_tensor(out=D1[:, a:b], in0=rt[:, a + 2: b + 2], in1=rt[:, a + 1: b + 1], op=ALU.subtract)
                nc.vector.tensor_scalar(out=D1s[:, a:b], in0=D1[:, a:b], scalar1=SINV, scalar2=None, op0=ALU.mult)
                nc.vector.tensor_tensor(out=DD[:, a:b], in0=D[:, a:b], in1=D1[:, a:b], op=ALU.mult)
                for c in range(3):
                    c1, c2 = (c + 1) % 3, (c + 2) % 3
                    nc.sync.dma_start(out=A1[16 * c: 16 * c + bpc, a:b], in_=Ds[16 * c1: 16 * c1 + bpc, a:b])
                    nc.sync.dma_start(out=A2[16 * c: 16 * c + bpc, a:b], in_=Ds[16 * c2: 16 * c2 + bpc, a:b])
                    nc.scalar.dma_start(out=B1[16 * c: 16 * c + bpc, a:b], in_=D1s[16 * c1: 16 * c1 + bpc, a:b])
                    nc.scalar.dma_start(out=B2[16 * c: 16 * c + bpc, a:b], in_=D1s[16 * c2: 16 * c2 + bpc, a:b])
                nc.vector.tensor_tensor(out=t_a[:, a:b], in0=A1[:, a:b], in1=B2[:, a:b], op=ALU.mult)
                nc.vector.tensor_tensor(out=t_b[:, a:b], in0=A2[:, a:b], in1=B1[:, a:b], op=ALU.mult)
                nc.vector.tensor_tensor(out=Cs[:, a:b], in0=t_a[:, a:b], in1=t_b[:, a:b], op=ALU.subtract)

            def emit_geo_b(h):
                a, b = rs(h, np_)
                for c in range(3):
                    nc.sync.dma_start(out=C1s[16 * c: 16 * c + bpc, a:b], in_=Cs[16 * c: 16 * c + bpc, a + 1: b + 1])
                nc.vector.tensor_tensor(out=XR[:, a:b], in0=Cs[:, a:b], in1=C1s[:, a:b], op=ALU.mult)
                nc.vector.tensor_tensor(out=YR[:, a:b], in0=Ds[:, a:b], in1=C1s[:, a:b], op=ALU.mult)
                a, b = rp(h, nl)
                lsqf = selmm2(DSQ, a, b, "lsq")
                nc.scalar.activation(out=lnl[:, a:b], in_=lsqf[:, 0: b - a], func=AF.Ln)
                nc.scalar.activation(out=rlen[:, a:b], in_=lnl[:, a:b], func=AF.Exp, scale=-0.5)
                nc.scalar.activation(out=lenf[:, a:b], in_=lnl[:, a:b], func=AF.Exp, scale=0.5)
                nc.scalar.activation(out=lenf16[:, a:b], in_=lnl[:, a:b], func=AF.Exp, scale=0.5, bias=lnsinv_t)
                sa, sb = rs(h, nl)
                nc.vector.tensor_scalar(out=F[0:16, sa:sb], in0=lenf[:, sa:sb], scalar1=3.8, scalar2=None, op0=ALU.subtract)
                sa, sb = rs(h, nt)
                nc.scalar.dma_start(out=rlen1[:, sa:sb], in_=rlen[:, sa + 1: sb + 1])
                sa, sb = rs(h, np_)
                nc.scalar.dma_start(out=lenf1[:, sa:sb], in_=lenf16[:, sa + 1: sb + 1])
                a, b = rp(h, nt)
                dotf = selmm2(DD, a, b, "dot")
                nc.vector.tensor_tensor(out=tt1[:, a:b], in0=dotf[:, 0: b - a], in1=rlen[:, a:b], op=ALU.mult)
                sa, sb = rs(h, nt)
                # +(d.d')*rlen_i*rlen_{i+1} = -cos(theta); sign folded into W1_ft
                nc.vector.tensor_tensor(out=F[32:48, sa:sb], in0=tt1[:, sa:sb], in1=rlen1[:, sa:sb], op=ALU.mult)

            def emit_geo_c(h):
                a, b = rs(h, np_)
                xrf = selmm2(XR, a, b, "xr")
                nc.scalar.activation(out=x_sb[:, a:b], in_=xrf[:, 0: b - a], func=AF.Copy)
                yrf = selmm2(YR, a, b, "yr")
                # y = (b1 . n2) * |b2| / 16 (the 1/16 rides in lenf16)
                nc.vector.tensor_tensor(out=y_sb[:, a:b], in0=yrf[:, 0: b - a], in1=lenf1[:, a:b], op=ALU.mult)
                nc.vector.tensor_tensor(out=q1[:, a:b], in0=x_sb[:, a:b], in1=x_sb[:, a:b], op=ALU.mult)
                nc.vector.tensor_tensor(out=q2[:, a:b], in0=y_sb[:, a:b], in1=y_sb[:, a:b], op=ALU.mult)
                nc.vector.tensor_tensor(out=q[:, a:b], in0=q1[:, a:b], in1=q2[:, a:b], op=ALU.add)
                nc.scalar.activation(out=lnq[:, a:b], in_=q[:, a:b], func=AF.Ln, bias=eps_t)
                nc.scalar.activation(out=r2v[:, a:b], in_=lnq[:, a:b], func=AF.Exp, scale=-0.5)
                # sin' = -sin(phi): sign folded into W1 sin rows host-side
                nc.vector.tensor_tensor(out=F[64:80, a:b], in0=y_sb[:, a:b], in1=r2v[:, a:b], op=ALU.mult)
                nc.vector.tensor_tensor(out=F[96:112, a:b], in0=x_sb[:, a:b], in1=r2v[:, a:b], op=ALU.mult)

            # ---------------- Phase 2: MLPs, two column-halves ----------------
            CH = {0: ((0, 512), (512, 511)), 1: ((SP, 512), (SP + 512, 512))}
            stack2_tiles = {}
            h1r_ref = {}

            def alloc_stack2(s):
                t = stk.tile([KROWS, ll], dt.float16, name="stack2", tag="stk")
                nc.sync.dma_start(out=t[0:65, SP:ll], in_=SE_d.ap()[s][:, SP:ll])
                stack2_tiles[s] = t

            def emit_w1(s, m, h):
                stack = (stack_tiles if h == 0 else stack2_tiles)[s]
                nv = ll - KOFF[m]
                h1 = h1_ps.tile([H, 2, W], dt.float32, name="h1", tag="h1ps")
                for ci, (c0, nmax) in enumerate(CH[h]):
                    n = min(nmax, nv - c0)
                    nc.tensor.matmul(h1[:, ci, :n], w1[m], stack[0:W1K[m], c0: c0 + n], start=True, stop=True)
                h1r = h1r_p.tile([H, 2, W], dt.float16, name="h1r", tag="h1r")
                nc.vector.tensor_scalar(
                    out=h1r.rearrange("p a b -> p (a b)"),
                    in0=h1.rearrange("p a b -> p (a b)"),
                    scalar1=0.0, scalar2=None, op0=ALU.max)
                h1r_ref[(s, m, h)] = h1r

            def emit_w2(s, m, h):
                nv = ll - KOFF[m]
                h1r = h1r_ref.pop((s, m, h))
                h2 = h2_ps.tile([H, 2, W], dt.float32, name="h2", tag="h2ps")
                nh = 0
                for ci, (c0, nmax) in enumerate(CH[h]):
                    n = min(nmax, nv - c0)
                    nc.tensor.matmul(h2[:, ci, :n], w2[m], h1r[:, ci, :n], start=True, stop=True)
                    nh = ci * W + n
                scr = scr_p.tile([H, 2, W], dt.float16, name="scr", tag="scr")
                nc.scalar.activation(
                    out=scr.rearrange("p a b -> p (a b)")[:, 0:nh],
                    in_=h2.rearrange("p a b -> p (a b)")[:, 0:nh],
                    func=AF.Relu, bias=b2c[m],
                    accum_out=acc[m][:, 2 * s + h: 2 * s + h + 1])

            Fv = F.rearrange("(f s) l -> f s l", s=32)

            def prep_sample(s, h):
                if h == 0:
                    if s not in stack_tiles:
                        alloc_stack(s)
                    if s + 2 < bpc and (s + 2) not in stack_tiles:
                        alloc_stack(s + 2)
                    stack = stack_tiles[s]
                    nc.sync.dma_start(out=stack[65:67, 0:SP], in_=Fv[0:2, s, 0:SP])
                    nc.sync.dma_start(out=stack[67:69, 0:SP], in_=Fv[2:4, s, 0:SP])
                else:
                    if s not in stack2_tiles:
                        alloc_stack2(s)
                    if s + 2 < bpc and (s + 2) not in stack2_tiles:
                        alloc_stack2(s + 2)
                    stack = stack2_tiles[s]
                    nc.sync.dma_start(out=stack[65:67, SP:ll], in_=Fv[0:2, s, SP:ll])
                    nc.sync.dma_start(out=stack[67:69, SP:ll], in_=Fv[2:4, s, SP:ll])

            emit_geo_a(0)
            emit_geo_b(0)
            emit_geo_c(0)

            for h in range(2):
                for s in range(bpc + 1):
                    if s < bpc:
                        prep_sample(s, h)
                    if h == 0 and s == 1:
                        emit_geo_a(1)
                    if h == 0 and s == 2:
                        emit_geo_b(1)
                    if h == 0 and s == 3:
                        emit_geo_c(1)
                    for m in MLPS:
                        if s < bpc:
                            emit_w1(s, m, h)
                        if s >= 1:
                            emit_w2(s - 1, m, h)

        # ---------------- final reduction ----------------
        with tc.tile_pool(name="fin_ps", bufs=1, space="PSUM") as fin_ps:
            ep = fin_ps.tile([1, 3, 2 * bpc], dt.float32, name="ep")
            for j, m in enumerate(MLPS):
                nc.tensor.matmul(ep[:, j, :], w3c[m], acc[m], start=True, stop=True)
            esum = consts.tile([1, bpc], dt.float32, name="esum")
            nc.vector.tensor_reduce(
                out=esum, in_=ep.rearrange("o m (s h) -> o s m h", h=2), axis=AX.XY, op=ALU.add)
            eout = consts.tile([1, bpc], dt.float32, name="eout")
            nc.vector.tensor_scalar(out=eout, in0=esum, scalar1=b3s, scalar2=None, op0=ALU.add)
            nc.sync.dma_start(out=out_d.ap(), in_=eout)

    nc.finalize()
    return nc


_NC_CACHE = {}


def get_nc(bpc=BPC, ll=L):
    key = (bpc, ll)
    if key not in _NC_CACHE:
        _NC_CACHE[key] = build_nc(bpc, ll)
    return _NC_CACHE[key]


def _sel48():
    S = np.zeros((48, 16), np.float16)
    for c in range(3):
        for s in range(16):
            S[16 * c + s, s] = 1.0
    return S


def pack_weights(inputs):
    """Pack per-MLP W1 into the [69, H] stack-row layout (fp16), fold b1 via
    the ones row, flip the torsion sin-row sign, fold the 3.8 len-centering
    into b1_fl."""
    f32 = lambda k: np.asarray(inputs[k], np.float32)
    W1P = np.zeros((3, KROWS, H), np.float32)
    # fl: x = [len, e0, e1]
    w = f32("fl_W1")
    W1P[0, 0:32] = w[1:33]
    W1P[0, 65] = w[0]
    W1P[0, 64] = f32("fl_b1") + 3.8 * w[0]
    # ft: x = [cos_t, e0, e1, e2]
    w = f32("ft_W1")
    W1P[1, 0:48] = w[1:49]
    W1P[1, 66] = -w[0]          # device stores -cos(theta)
    W1P[1, 64] = f32("ft_b1")
    # fp: x = [sin, cos, e0, e1, e2, e3]
    w = f32("fp_W1")
    W1P[2, 0:64] = w[2:66]
    W1P[2, 67] = -w[0]          # device computes -sin
    W1P[2, 68] = w[1]
    W1P[2, 64] = f32("fp_b1")
    W2P = np.stack([f32(f"{m}_W2") for m in MLPS]).astype(np.float16)
    B2P = np.stack([f32(f"{m}_b2").reshape(H, 1) for m in MLPS]).astype(np.float32)
    W3P = np.stack([f32(f"{m}_W3") for m in MLPS]).astype(np.float32)
    b3sum = np.float32(
        float(np.asarray(inputs["fl_b3"]).reshape(-1)[0]) * NL
        + float(np.asarray(inputs["ft_b3"]).reshape(-1)[0]) * NT
        + float(np.asarray(inputs["fp_b3"]).reshape(-1)[0]) * NP
    )
    return W1P.astype(np.float16), W2P, B2P, W3P, np.array([[b3sum]], np.float32)


def make_in_maps(inputs, bpc=BPC, ncores=NCORES):
    W1P, W2P, B2P, W3P, B3S = pack_weights(inputs)
    WALL = np.zeros((H, 6 * H), np.float16)
    for j in range(3):
        WALL[0:KROWS, H * j: H * (j + 1)] = W1P[j]
        WALL[:, H * (3 + j): H * (4 + j)] = W2P[j]
    BW = np.zeros((H, 8), np.float32)
    for j in range(3):
        BW[:, j] = B2P[j][:, 0]
        BW[:, 3 + j] = W3P[j][:, 0]
    BW[0, 6] = B3S[0, 0]
    emb16 = np.asarray(inputs["emb"], np.float32).astype(np.float16)
    seq = np.asarray(inputs["seq"], np.int64)
    R = np.asarray(inputs["R"], np.float32)
    e_all = emb16[seq]                       # [B, L, E]
    consts = dict(S48=_sel48(), WALL=WALL, BW=BW)
    in_maps = []
    for c in range(ncores):
        sl = slice(c * bpc, (c + 1) * bpc)
        Rt = np.ascontiguousarray(R[sl].transpose(2, 0, 1))        # [3, bpc, L]
        e = e_all[sl]                                              # [bpc, L, E]
        SE = np.zeros((bpc, 65, L), np.float16)
        for j in range(4):
            SE[:, 16 * j: 16 * j + 16, : L - j] = e[:, j:, :].transpose(0, 2, 1)
        SE[:, 64, :] = 1.0
        m = dict(consts)
        m["Rt"] = Rt
        m["SE"] = SE
        in_maps.append(m)
    return in_maps


def kernel(**inputs):
    nc = get_nc()
    in_maps = make_in_maps(inputs)
    res = bass_utils.run_bass_kernel_spmd(nc, in_maps, core_ids=list(range(NCORES)))
    return np.concatenate([res.results[c]["out"][0] for c in range(NCORES)]).astype(np.float32)
